# revision 16
# baseline (speedup 1.0000x reference)
"""Trainium2 Bass kernel for nn_DPASSMBlock (sliding-window attn + SSM + MLP).

Sharding: sequence-parallel over 8 cores. Core c handles batch b=c//4,
token chunk s=c%4 of 512 tokens, with a 256-token halo (recomputed K/V/u).
No collectives. Activations live in transposed layout [d_part, tok_free].
Matmuls in bf16; LN stats / softmax pointwise / SSM scan / residual in fp32.
"""
import numpy as np
import ml_dtypes
from contextlib import ExitStack

import concourse.bass as bass
import concourse.mybir as mybir
import concourse.tile as tile
from concourse import bacc
from concourse.bass_utils import run_bass_kernel_spmd

F32 = mybir.dt.float32
BF16 = mybir.dt.bfloat16
AF = mybir.ActivationFunctionType
ALU = mybir.AluOpType

P = 128
D = 1024
DT = 8            # d-tiles
T_OWN = 512
HALO = 256
T_TOT = 768       # halo + own
NH = 16
DH = 64
SSM_N = 256
HID = 4096
HT = 32           # hidden tiles
EPS = 1e-5

# scores band structure: k-tile j serves own-q range [QS[j], QE[j])
QS = [0, 0, 0, 128, 256, 384]
QE = [128, 256, 384, 512, 512, 512]
NJ = [QE[j] - QS[j] for j in range(6)]          # 128,256,384,384,256,128
MOFF = np.cumsum([0] + [2 * n for n in NJ]).tolist()  # maskpack offsets

# biaspack column map
BQ, BK, BV, BG, BO, B1, B2 = 0, 8, 16, 24, 32, 40, 72
L1G, L1B, L2G, L2B = 80, 88, 96, 104
ALP, SINJ, UHF, EPSC = 112, 114, 116, 117
BPCOLS = 118

_CACHE = {}


def build():
    nc = bacc.Bacc("TRN2", target_bir_lowering=False, debug=False)
    dram = {}

    def din(name, shape, dt=F32):
        dram[name] = nc.dram_tensor(name, shape, dt, kind="ExternalInput")
        return dram[name]

    xT = din("xT", [D, T_TOT])
    wq = din("wq", [DT, P, D], BF16)
    wk = din("wk", [DT, P, D], BF16)
    wv = din("wv", [DT, P, D], BF16)
    wg = din("wg", [DT, P, D], BF16)
    wo = din("wo", [DT, P, D], BF16)
    bw = din("bw", [DT, P, SSM_N], BF16)
    cw = din("cw", [2, P, D], BF16)
    w1 = din("w1", [HT, DT, P, P], BF16)
    w2 = din("w2", [2, HT, P, 4 * P], BF16)
    bp_d = din("biaspack", [P, BPCOLS])
    mk_d = din("maskpack", [P, MOFF[6]], BF16)
    outT = nc.dram_tensor("outT", [D, T_OWN], F32, kind="ExternalOutput")
    out_state = nc.dram_tensor("out_state", [2, P, 1], F32, kind="ExternalOutput")

    with tile.TileContext(nc) as tc:
        with ExitStack() as ctx:
            def mkpool(name, side=None):
                es = ExitStack()
                pool = es.enter_context(
                    tc.tile_pool(name=name, bufs=1, side=side))
                return pool, es
            sb = ctx.enter_context(tc.tile_pool(name="sb", bufs=1, side="left"))
            wpool = ctx.enter_context(tc.tile_pool(name="wpool", bufs=1, side="left"))
            x_p, x_es = mkpool("x_p", "left")

            # ---- constants ----
            bp = sb.tile([P, BPCOLS], F32, tag="bp")
            nc.sync.dma_start(bp[:], bp_d.ap())
            mk = sb.tile([P, MOFF[6]], BF16, tag="mk")
            nc.sync.dma_start(mk[:], mk_d.ap())
            ones_bf = sb.tile([P, 1], BF16, tag="ones_bf")
            nc.vector.memset(ones_bf[:], 1.0)
            ones_mat = sb.tile([P, P], BF16, tag="ones_mat")
            nc.vector.memset(ones_mat[:], 1.0)

            # ---- load xT ----
            xt = []
            for j in range(DT):
                t = x_p.tile([P, T_TOT], F32, tag=f"xt{j}", name=f"xt{j}")
                nc.sync.dma_start(t[:], xT.ap()[j * P:(j + 1) * P, :])
                xt.append(t)

            # ================= LN1 (stats via ones-matmuls) =================
            def layernorm(src, tok_len, gcol, bcol, out_dt, out_tag, dst):
                """src: list of 8 [P, tok_len] f32 tiles -> 8 [P, tok_len] out_dt."""
                with tc.tile_pool(name=f"ps_{out_tag}", bufs=1, space="PSUM") as pp, \
                     tc.tile_pool(name=f"ln_{out_tag}", bufs=1) as lp:
                    nhalf = tok_len // 2
                    s_x = [pp.tile([1, nhalf], F32, tag=f"sx{h}", name=f"sx{h}_{out_tag}")
                           for h in range(2)]
                    s_q = [pp.tile([1, nhalf], F32, tag=f"sq{h}", name=f"sq{h}_{out_tag}")
                           for h in range(2)]
                    for j in range(DT):
                        xb = lp.tile([P, tok_len], BF16, tag="ln_xb", bufs=2,
                                     name=f"xb{j}_{out_tag}")
                        nc.vector.tensor_copy(xb[:], src[j][:])
                        xq = lp.tile([P, tok_len], BF16, tag="ln_xq", bufs=2,
                                     name=f"xq{j}_{out_tag}")
                        nc.scalar.square(xq[:], xb[:])
                        for h in range(2):
                            sl = slice(h * nhalf, (h + 1) * nhalf)
                            nc.tensor.matmul(s_x[h][:], ones_bf[:], xb[:, sl],
                                             start=(j == 0), stop=(j == DT - 1))
                            nc.tensor.matmul(s_q[h][:], ones_bf[:], xq[:, sl],
                                             start=(j == 0), stop=(j == DT - 1))
                    # stats -> scale a[t]=rstd, bias b[t]=-mean*rstd  (on [1,tok])
                    rows = lp.tile([1, 4 * tok_len], F32, tag="ln_rows",
                                   bufs=1, name=f"rows_{out_tag}")
                    mean = rows[:, 0:tok_len]
                    var = rows[:, tok_len:2 * tok_len]
                    a_row = rows[:, 2 * tok_len:3 * tok_len]
                    b_row = rows[:, 3 * tok_len:4 * tok_len]
                    for h in range(2):
                        sl = slice(h * nhalf, (h + 1) * nhalf)
                        nc.vector.tensor_scalar_mul(mean[:, sl], s_x[h][:], 1.0 / D)
                        # var = sq/D - mean^2  (+eps folded into sqrt bias)
                        nc.scalar.square(var[:, sl], mean[:, sl])
                        nc.vector.scalar_tensor_tensor(
                            var[:, sl], s_q[h][:], 1.0 / D, var[:, sl],
                            ALU.mult, ALU.subtract)
                    nc.scalar.activation(a_row[:], var[:], AF.Sqrt,
                                         bias=bp[0:1, EPSC:EPSC + 1])
                    nc.vector.reciprocal_approx_fast(out=a_row[:], in_=a_row[:])
                    nc.vector.scalar_tensor_tensor(
                        b_row[:], mean[:], -1.0, a_row[:], ALU.mult, ALU.mult)
                    # broadcast rows across partitions
                    a_b = lp.tile([P, tok_len], F32, tag="ln_ab", bufs=1,
                                  name=f"ab_{out_tag}")
                    b_b = lp.tile([P, tok_len], F32, tag="ln_bb", bufs=1,
                                  name=f"bb_{out_tag}")
                    nc.gpsimd.partition_broadcast(a_b[:], a_row[:])
                    nc.gpsimd.partition_broadcast(b_b[:], b_row[:])
                    outs = []
                    for j in range(DT):
                        t1 = lp.tile([P, tok_len], F32, tag="ln_t1", bufs=2,
                                     name=f"t1_{j}_{out_tag}")
                        nc.vector.tensor_mul(t1[:], src[j][:], a_b[:])
                        nc.vector.tensor_add(t1[:], t1[:], b_b[:])
                        o = dst.tile([P, tok_len], out_dt, tag=f"{out_tag}{j}",
                                     name=f"{out_tag}{j}")
                        nc.scalar.activation(o[:], t1[:], AF.Identity,
                                             bias=bp[:, bcol + j:bcol + j + 1],
                                             scale=bp[:, gcol + j:gcol + j + 1])
                        outs.append(o)
                    return outs

            ssm_p, ssm_es = mkpool("ssm_p", "right")
            qkv_p, qkv_es = mkpool("qkv_p", "right")
            xn_p, xn_es = mkpool("xn_p", "right")
            gate_p, gate_es = mkpool("gate_p", "left")
            xn = layernorm(xt, T_TOT, L1G, L1B, BF16, "xn", xn_p)

            # ================= projections =================
            def load_w(dram_t, j, tag, bufs, cols=D):
                t = wpool.tile([P, cols], BF16, tag=tag, bufs=bufs,
                               name=f"{tag}_{dram_t.name}_{j}")
                nc.sync.dma_start(t[:], dram_t.ap()[j])
                return t

            def proj_T(wname, dram_t, tok0, tok1, bias_base, scale, out_dt,
                       douts=DT, out_len=None, pp=None, wcols=D, dst=None):
                """Transposed-out projection: out[dout_tile][P, tok1-tok0]."""
                outs = []
                wtiles = [load_w(dram_t, j, "wgt", 12, cols=wcols)
                          for j in range(DT)]
                out_len = out_len or (tok1 - tok0)
                for d in range(douts):
                    o = dst.tile([P, out_len], out_dt, tag=f"{wname}{d}",
                                 name=f"{wname}{d}")
                    for blk0 in range(0, out_len, 512):
                        blk1 = min(blk0 + 512, out_len)
                        ps = pp.tile([P, blk1 - blk0], F32, tag="proj", bufs=4,
                                     name=f"ps_{wname}{d}_{blk0}")
                        for j in range(DT):
                            nc.tensor.matmul(
                                ps[:], wtiles[j][:, d * P:(d + 1) * P],
                                xn[j][:, tok0 + blk0:tok0 + blk1],
                                start=(j == 0), stop=(j == DT - 1))
                        if bias_base is None:
                            nc.scalar.copy(o[:, blk0:blk1], ps[:])
                        else:
                            nc.scalar.activation(
                                o[:, blk0:blk1], ps[:], AF.Identity,
                                bias=bp[:, bias_base + d:bias_base + d + 1],
                                scale=scale)
                    outs.append(o)
                return outs

            with tc.tile_pool(name="ps_proj", bufs=1, space="PSUM") as pp:
                kt = proj_T("kt", wk, 0, T_TOT, BK, 1.0, BF16, pp=pp, dst=qkv_p)
                qt = proj_T("qt", wq, HALO, T_TOT, BQ, 0.125, BF16, pp=pp, dst=qkv_p)
                # u projection (SSM input), fp32 out, no bias
                ut = proj_T("ut", bw, 0, T_TOT, None, 1.0, F32, douts=2,
                            pp=pp, wcols=SSM_N, dst=ssm_p)
                # V in natural layout [tok_part, head*64]
                wvt = [load_w(wv, j, "wgt", 12) for j in range(DT)]
                vp = []
                for t_i in range(6):
                    v = qkv_p.tile([P, D], BF16, tag=f"vp{t_i}", name=f"vp{t_i}")
                    for b in range(2):
                        ps = pp.tile([P, 512], F32, tag="proj", bufs=4,
                                     name=f"ps_v{t_i}_{b}")
                        for j in range(DT):
                            nc.tensor.matmul(
                                ps[:], xn[j][:, t_i * P:(t_i + 1) * P],
                                wvt[j][:, b * 512:(b + 1) * 512],
                                start=(j == 0), stop=(j == DT - 1))
                        nc.scalar.copy(v[:, b * 512:(b + 1) * 512], ps[:])
                    vp.append(v)
                wgt_t = [load_w(wg, j, "wgt", 12) for j in range(DT)]
                gs = []
                for d in range(DT):
                    g = gate_p.tile([P, T_OWN], BF16, tag=f"gs{d}",
                                    name=f"gs{d}")
                    ps = pp.tile([P, T_OWN], F32, tag="proj", bufs=4,
                                 name=f"ps_g{d}")
                    for j in range(DT):
                        nc.tensor.matmul(ps[:], wgt_t[j][:, d * P:(d + 1) * P],
                                         xn[j][:, HALO:T_TOT],
                                         start=(j == 0), stop=(j == DT - 1))
                    nc.scalar.activation(g[:], ps[:], AF.Sigmoid,
                                         bias=bp[:, BG + d:BG + d + 1])
                    gs.append(g)
                xn_es.close()

            # ================= SSM scan =================
            states = []
            for i in range(2):
                # chunk-0 cores: zero the halo-u (nonzero via ln1_b), then
                # inject the initial state as a virtual token at halo col 255
                nc.vector.tensor_scalar_mul(
                    ut[i][:, 0:HALO], ut[i][:, 0:HALO],
                    bp[:, UHF:UHF + 1])
                nc.vector.tensor_scalar_add(
                    ut[i][:, HALO - 1:HALO], ut[i][:, HALO - 1:HALO],
                    bp[:, SINJ + i:SINJ + i + 1])
                ar = ssm_p.tile([P, T_TOT], F32, tag=f"ar{i}", name=f"ar{i}")
                nc.vector.memset(ar[:], 1.0)
                nc.vector.tensor_scalar_mul(ar[:], ar[:],
                                            bp[:, ALP + i:ALP + i + 1])
                st = ssm_p.tile([P, T_TOT], F32, tag=f"st{i}", name=f"st{i}")
                nc.vector.tensor_tensor_scan(st[:], ar[:], ut[i][:], 0.0,
                                             ALU.mult, ALU.add)
                nc.sync.dma_start(out_state.ap()[i], st[:, T_TOT - 1:T_TOT])
                sbf = ssm_p.tile([P, T_OWN], BF16, tag=f"sbf{i}", name=f"sbf{i}")
                nc.vector.tensor_copy(sbf[:], st[:, HALO:T_TOT])
                states.append(sbf)

            # ================= attention =================
            pt_p, pt_es = mkpool("pt_p", "right")
            aot_p, aot_es = mkpool("aot_p", "left")
            aot = [aot_p.tile([P, T_OWN], BF16, tag=f"aot{t}", name=f"aot{t}")
                   for t in range(DT)]
            pt = {}

            def scores_stage(j, pa):
                ptile = pt_p.tile([P, NH * NJ[j]], BF16, tag="pt", bufs=3,
                                padded_shape=[P, NH * 384], name=f"pt{j}")
                pt[j] = ptile
                n = NJ[j]
                for h in range(NH):
                    t, half = h // 2, h % 2
                    ps = pa.tile([P, n], F32, tag="sc", bufs=3,
                                 name=f"ps_sc{j}_{h}")
                    nc.tensor.matmul(
                        ps[:],
                        kt[t][half * DH:(half + 1) * DH, j * P:(j + 1) * P],
                        qt[t][half * DH:(half + 1) * DH, QS[j]:QE[j]],
                        start=True, stop=True)
                    nc.scalar.activation(ptile[:, h * n:(h + 1) * n], ps[:],
                                         AF.Exp)
                for hp in range(NH // 2):
                    sl = slice(2 * hp * n, (2 * hp + 2) * n)
                    nc.vector.tensor_mul(ptile[:, sl], ptile[:, sl],
                                         mk[:, MOFF[j]:MOFF[j] + 2 * n])

            def pv_stage(s, pa):
                for pr in range(8):
                    h0 = 2 * pr
                    pso = pa.tile([P, 2 * P], F32, tag="ao", bufs=3,
                                  name=f"ps_ao{s}_{pr}")
                    psd = pa.tile([P, 2 * P], F32, tag="den", bufs=2,
                                  name=f"ps_den{s}_{pr}")
                    for i, j in enumerate((s, s + 1, s + 2)):
                        n = NJ[j]
                        off = 128 * s - QS[j]
                        rhs = pt[j][:].rearrange("p (h q) -> p h q", q=n)[
                            :, h0:h0 + 2, off:off + P]
                        nc.tensor.matmul(pso[:], vp[j][:, DH * h0:DH * h0 + 128],
                                         rhs, start=(i == 0), stop=(i == 2))
                        nc.tensor.matmul(psd[:], ones_mat[:], rhs,
                                         start=(i == 0), stop=(i == 2))
                    recb = pt_p.tile([P, 2 * P], F32, tag="recb", bufs=3,
                                     name=f"recb{s}_{pr}")
                    nc.vector.reciprocal_approx_fast(out=recb[:], in_=psd[:])
                    nc.vector.tensor_mul(aot[pr][0:DH, 128 * s:128 * (s + 1)],
                                         pso[0:DH, 0:P], recb[0:DH, 0:P])
                    nc.vector.tensor_mul(aot[pr][DH:P, 128 * s:128 * (s + 1)],
                                         pso[DH:P, P:2 * P], recb[DH:P, P:2 * P])

            with tc.tile_pool(name="ps_attn", bufs=1, space="PSUM") as pa:
                # Vp slot for pair pr: cols [65*2pr, 65*2pr+130) -> slice 128 of it
                scores_stage(0, pa)
                scores_stage(1, pa)
                scores_stage(2, pa)
                for s in range(4):
                    pv_stage(s, pa)
                    if s + 3 <= 5:
                        scores_stage(s + 3, pa)
            pt_es.close()
            qkv_es.close()

            # ================= wo projection + y_ssm =================
            ya_p, ya_es = mkpool("ya_p", "left")
            with tc.tile_pool(name="ps_post", bufs=1, space="PSUM") as pp:
                wotiles = [load_w(wo, j, "wgt", 12) for j in range(DT)]
                ya = []
                for d in range(DT):
                    ps = pp.tile([P, T_OWN], F32, tag="proj", bufs=4,
                                 name=f"ps_wo{d}")
                    for t in range(DT):
                        nc.tensor.matmul(ps[:], wotiles[t][:, d * P:(d + 1) * P],
                                         aot[t][:], start=(t == 0),
                                         stop=(t == DT - 1))
                    y = ya_p.tile([P, T_OWN], F32, tag=f"ya{d}", name=f"ya{d}")
                    nc.scalar.activation(y[:], ps[:], AF.Identity,
                                         bias=bp[:, BO + d:BO + d + 1])
                    ya.append(y)
                cwtiles = [load_w(cw, i, "wgt", 12) for i in range(2)]
                ys = []
                for d in range(DT):
                    ps = pp.tile([P, T_OWN], F32, tag="proj", bufs=4,
                                 name=f"ps_cw{d}")
                    for i in range(2):
                        nc.tensor.matmul(ps[:], cwtiles[i][:, d * P:(d + 1) * P],
                                         states[i][:], start=(i == 0),
                                         stop=(i == 1))
                    y = ya_p.tile([P, T_OWN], F32, tag=f"ys{d}", name=f"ys{d}")
                    nc.scalar.copy(y[:], ps[:])
                    ys.append(y)

            # ================= gated fusion + residual =================
            xmid = []
            for d in range(DT):
                t1 = sb.tile([P, T_OWN], F32, tag="fus_t", bufs=3,
                             name=f"fus{d}")
                nc.vector.tensor_sub(t1[:], ya[d][:], ys[d][:])
                nc.vector.tensor_mul(t1[:], t1[:], gs[d][:])
                nc.vector.tensor_add(t1[:], t1[:], ys[d][:])
                xm = sb.tile([P, T_OWN], F32, tag=f"xmid{d}", name=f"xmid{d}")
                nc.vector.tensor_add(xm[:], t1[:], xt[d][:, HALO:T_TOT])
                xmid.append(xm)
            ssm_es.close()
            ya_es.close()
            aot_es.close()
            gate_es.close()
            x_es.close()

            # ================= LN2 + MLP =================
            with tc.tile_pool(name="mlp_p", bufs=1, side="left") as mlp_p:
                x3 = layernorm(xmid, T_OWN, L2G, L2B, BF16, "x3", mlp_p)
                with tc.tile_pool(name="ps_mlp", bufs=1, space="PSUM") as pm:
                    ht = []
                    for h in range(HT):
                        w1t = wpool.tile([P, D], BF16, tag="w1t", bufs=3,
                                         name=f"w1t{h}")
                        nc.sync.dma_start(
                            w1t[:].rearrange("p (j m) -> p j m", m=P),
                            w1.ap()[h].rearrange("j p m -> p j m"))
                        ps = pm.tile([P, T_OWN], F32, tag="h", bufs=3,
                                     name=f"ps_h{h}")
                        for j in range(DT):
                            nc.tensor.matmul(ps[:], w1t[:, j * P:(j + 1) * P],
                                             x3[j][:], start=(j == 0),
                                             stop=(j == DT - 1))
                        o = mlp_p.tile([P, T_OWN], BF16, tag="ht", bufs=HT,
                                       name=f"ht{h}")
                        nc.scalar.activation(o[:], ps[:], AF.Gelu,
                                             bias=bp[:, B1 + h:B1 + h + 1])
                        ht.append(o)
                    for half in range(2):
                        psd = [pm.tile([P, T_OWN], F32, tag=f"o{d4}", bufs=1,
                                       name=f"ps_o{half}_{d4}")
                               for d4 in range(4)]
                        for j in range(HT):
                            w2t = wpool.tile([P, 4 * P], BF16, tag="w2t",
                                             bufs=3, name=f"w2t{half}_{j}")
                            nc.sync.dma_start(w2t[:], w2.ap()[half, j])
                            for d4 in range(4):
                                nc.tensor.matmul(psd[d4][:],
                                                 w2t[:, d4 * P:(d4 + 1) * P],
                                                 ht[j][:], start=(j == 0),
                                                 stop=(j == HT - 1))
                        for d4 in range(4):
                            d = 4 * half + d4
                            o = mlp_p.tile([P, T_OWN], F32, tag="oT", bufs=3,
                                           name=f"oT{d}")
                            nc.vector.scalar_tensor_tensor(
                                o[:], psd[d4][:], bp[:, B2 + d:B2 + d + 1],
                                xmid[d][:], ALU.add, ALU.add)
                            nc.sync.dma_start(outT.ap()[d * P:(d + 1) * P, :],
                                              o[:])
    nc.compile()
    return nc


def _masks(chunk0: bool) -> np.ndarray:
    jj = np.arange(P)[:, None]
    mask = np.zeros((P, MOFF[6]), np.float32)
    for j in range(6):
        segs = []
        for s in range(max(0, j - 2), min(3, j) + 1):
            qq = np.arange(128)[None, :]
            if s == j:          # tri-A: key strictly after q
                m = (jj > qq)
            elif s == j - 1:    # fully valid
                m = np.ones((P, 128), bool)
            else:               # s == j - 2: causal
                m = (jj <= qq)
            segs.append(m.astype(np.float32))
        mj = np.concatenate(segs, axis=1)
        if chunk0 and j < 2:
            mj = np.zeros_like(mj)
        mask[:, MOFF[j]:MOFF[j] + 2 * NJ[j]] = np.concatenate([mj, mj], axis=1)
    return mask.astype(ml_dtypes.bfloat16)


def kernel(x, ssm_state, ln1_g, ln1_b, ln2_g, ln2_b, wq, bq, wk, bk, wv, bv,
           wo, bo, wg, bg, A, Bw, Cw, w1, b1, w2, b2):
    if "nc" not in _CACHE:
        _CACHE["nc"] = build()
    nc = _CACHE["nc"]

    bf = ml_dtypes.bfloat16

    def wl(w):      # [din, dout] -> [din/128, 128, dout] bf16
        return np.ascontiguousarray(w.reshape(DT, P, -1).astype(bf))

    wq_l, wk_l, wv_l, wg_l, wo_l = wl(wq), wl(wk), wl(wv), wl(wg), wl(wo)
    bw_l = np.ascontiguousarray(Bw.reshape(DT, P, SSM_N).astype(bf))
    cw_l = np.ascontiguousarray(Cw.reshape(2, P, D).astype(bf))
    w1_l = np.ascontiguousarray(
        w1.reshape(DT, P, HT, P).transpose(2, 0, 1, 3).astype(bf))
    w2_l = np.ascontiguousarray(
        w2.reshape(HT, P, 2, 512).transpose(2, 0, 1, 3).astype(bf))

    bo_eff = (bv.astype(np.float64) @ wo.astype(np.float64)
              + bo.astype(np.float64)).astype(np.float32)
    alpha = np.tanh(A.astype(np.float64)).astype(np.float32)

    bp = np.zeros((P, BPCOLS), np.float32)

    def fill(col, vec):
        v = vec.reshape(-1, P)
        for i in range(v.shape[0]):
            bp[:, col + i] = v[i]

    fill(BQ, bq * 0.125)
    fill(BK, bk)
    fill(BV, bv * 0)          # unused (folded into bo_eff)
    fill(BG, bg)
    fill(BO, bo_eff)
    fill(B1, b1)
    fill(B2, b2)
    fill(L1G, ln1_g)
    fill(L1B, ln1_b)
    fill(L2G, ln2_g)
    fill(L2B, ln2_b)
    fill(ALP, alpha)
    bp[:, EPSC] = EPS

    mask_c0 = _masks(True)
    mask_in = _masks(False)

    in_maps = []
    for c in range(8):
        b, s = c // 4, c % 4
        xfull = np.zeros((T_TOT, D), np.float32)
        t0 = s * T_OWN
        if s > 0:
            xfull[:HALO] = x[b, t0 - HALO:t0]
        xfull[HALO:] = x[b, t0:t0 + T_OWN]
        bpc = bp.copy()
        bpc[:, UHF] = 0.0 if s == 0 else 1.0
        if s == 0:
            st = np.asarray(ssm_state[b]).reshape(2, P)
            bpc[:, SINJ] = st[0]
            bpc[:, SINJ + 1] = st[1]
        in_maps.append({
            "xT": np.ascontiguousarray(xfull.T),
            "wq": wq_l, "wk": wk_l, "wv": wv_l, "wg": wg_l, "wo": wo_l,
            "bw": bw_l, "cw": cw_l, "w1": w1_l, "w2": w2_l,
            "biaspack": bpc,
            "maskpack": mask_c0 if s == 0 else mask_in,
        })

    res = run_bass_kernel_spmd(nc, in_maps, core_ids=list(range(8)))
    _CACHE["last_results"] = res

    x_out = np.zeros((2, 2048, D), np.float32)
    new_state = np.zeros((2, SSM_N), np.float32)
    for c in range(8):
        b, s = c // 4, c % 4
        x_out[b, s * T_OWN:(s + 1) * T_OWN] = res.results[c]["outT"].T
        if s == 3:
            new_state[b] = res.results[c]["out_state"].reshape(SSM_N)
    return x_out, new_state


# revision 18
# speedup vs baseline: 1.1543x; 1.1543x over previous
"""Trainium2 Bass kernel for nn_DPASSMBlock (sliding-window attn + SSM + MLP).

Sharding: sequence-parallel over 8 cores. Core c handles batch b=c//4,
token chunk s=c%4 of 512 tokens, with a 256-token halo (recomputed K/V/u).
No collectives. Activations live in transposed layout [d_part, tok_free].
Matmuls in bf16; LN stats / softmax pointwise / SSM scan / residual in fp32.
"""
import numpy as np
import ml_dtypes
from contextlib import ExitStack

import concourse.bass as bass
import concourse.mybir as mybir
import concourse.tile as tile
from concourse import bacc
from concourse.bass_utils import run_bass_kernel_spmd

F32 = mybir.dt.float32
BF16 = mybir.dt.bfloat16
AF = mybir.ActivationFunctionType
ALU = mybir.AluOpType

P = 128
D = 1024
DT = 8            # d-tiles
T_OWN = 512
HALO = 256
T_TOT = 768       # halo + own
NH = 16
DH = 64
SSM_N = 256
HID = 4096
HT = 32           # hidden tiles
EPS = 1e-5

# scores band structure: k-tile j serves own-q range [QS[j], QE[j])
QS = [0, 0, 0, 128, 256, 384]
QE = [128, 256, 384, 512, 512, 512]
NJ = [QE[j] - QS[j] for j in range(6)]          # 128,256,384,384,256,128
MOFF = np.cumsum([0] + [2 * n for n in NJ]).tolist()  # maskpack offsets

# biaspack column map
BQ, BK, BV, BG, BO, B1, B2 = 0, 8, 16, 24, 32, 40, 72
L1G, L1B, L2G, L2B = 80, 88, 96, 104
ALP, SINJ, UHF, EPSC = 112, 114, 116, 117
BPCOLS = 118

_CACHE = {}


def build():
    nc = bacc.Bacc("TRN2", target_bir_lowering=False, debug=False)
    dram = {}

    def din(name, shape, dt=F32):
        dram[name] = nc.dram_tensor(name, shape, dt, kind="ExternalInput")
        return dram[name]

    xT = din("xT", [D, T_TOT])
    wq = din("wq", [DT, P, D], BF16)
    wk = din("wk", [DT, P, D], BF16)
    wv = din("wv", [DT, P, D], BF16)
    wg = din("wg", [DT, P, D], BF16)
    wo = din("wo", [DT, P, D], BF16)
    bw = din("bw", [DT, P, SSM_N], BF16)
    cw = din("cw", [2, P, D], BF16)
    w1 = din("w1", [HT, DT, P, P], BF16)
    w2 = din("w2", [2, HT, P, 4 * P], BF16)
    bp_d = din("biaspack", [P, BPCOLS])
    mk_d = din("maskpack", [P, MOFF[6]], BF16)
    outT = nc.dram_tensor("outT", [D, T_OWN], F32, kind="ExternalOutput")
    out_state = nc.dram_tensor("out_state", [2, P, 1], F32, kind="ExternalOutput")

    with tile.TileContext(nc) as tc:
        with ExitStack() as ctx:
            def mkpool(name, side=None):
                es = ExitStack()
                pool = es.enter_context(
                    tc.tile_pool(name=name, bufs=1, side=side))
                return pool, es
            sb = ctx.enter_context(tc.tile_pool(name="sb", bufs=1, side="left"))
            wpool = ctx.enter_context(tc.tile_pool(name="wpool", bufs=1, side="left"))
            x_p, x_es = mkpool("x_p", "left")

            # ---- constants ----
            bp = sb.tile([P, BPCOLS], F32, tag="bp")
            nc.sync.dma_start(bp[:], bp_d.ap())
            mk = sb.tile([P, MOFF[6]], BF16, tag="mk")
            nc.sync.dma_start(mk[:], mk_d.ap())
            ones_bf = sb.tile([P, 1], BF16, tag="ones_bf")
            nc.vector.memset(ones_bf[:], 1.0)
            ones_mat = sb.tile([P, P], BF16, tag="ones_mat")
            nc.vector.memset(ones_mat[:], 1.0)

            # ---- load xT ----
            xt = []
            for j in range(DT):
                t = x_p.tile([P, T_TOT], F32, tag=f"xt{j}", name=f"xt{j}")
                nc.sync.dma_start(t[:], xT.ap()[j * P:(j + 1) * P, :])
                xt.append(t)

            # ================= LN1 (stats via ones-matmuls) =================
            def layernorm(src, tok_len, gcol, bcol, out_dt, out_tag, dst):
                """src: list of 8 [P, tok_len] f32 tiles -> 8 [P, tok_len] out_dt."""
                with tc.tile_pool(name=f"ps_{out_tag}", bufs=1, space="PSUM") as pp, \
                     tc.tile_pool(name=f"ln_{out_tag}", bufs=1) as lp:
                    nhalf = tok_len // 2
                    s_x = [pp.tile([1, nhalf], F32, tag=f"sx{h}", name=f"sx{h}_{out_tag}")
                           for h in range(2)]
                    s_q = [pp.tile([1, nhalf], F32, tag=f"sq{h}", name=f"sq{h}_{out_tag}")
                           for h in range(2)]
                    for j in range(DT):
                        xb = lp.tile([P, tok_len], BF16, tag="ln_xb", bufs=2,
                                     name=f"xb{j}_{out_tag}")
                        nc.vector.tensor_copy(xb[:], src[j][:])
                        xq = lp.tile([P, tok_len], BF16, tag="ln_xq", bufs=2,
                                     name=f"xq{j}_{out_tag}")
                        nc.scalar.square(xq[:], xb[:])
                        for h in range(2):
                            sl = slice(h * nhalf, (h + 1) * nhalf)
                            nc.tensor.matmul(s_x[h][:], ones_bf[:], xb[:, sl],
                                             start=(j == 0), stop=(j == DT - 1))
                            nc.tensor.matmul(s_q[h][:], ones_bf[:], xq[:, sl],
                                             start=(j == 0), stop=(j == DT - 1))
                    # stats -> scale a[t]=rstd, bias b[t]=-mean*rstd  (on [1,tok])
                    rows = lp.tile([1, 4 * tok_len], F32, tag="ln_rows",
                                   bufs=1, name=f"rows_{out_tag}")
                    mean = rows[:, 0:tok_len]
                    var = rows[:, tok_len:2 * tok_len]
                    a_row = rows[:, 2 * tok_len:3 * tok_len]
                    b_row = rows[:, 3 * tok_len:4 * tok_len]
                    for h in range(2):
                        sl = slice(h * nhalf, (h + 1) * nhalf)
                        nc.vector.tensor_scalar_mul(mean[:, sl], s_x[h][:], 1.0 / D)
                        # var = sq/D - mean^2  (+eps folded into sqrt bias)
                        nc.scalar.square(var[:, sl], mean[:, sl])
                        nc.vector.scalar_tensor_tensor(
                            var[:, sl], s_q[h][:], 1.0 / D, var[:, sl],
                            ALU.mult, ALU.subtract)
                    nc.scalar.activation(a_row[:], var[:], AF.Sqrt,
                                         bias=bp[0:1, EPSC:EPSC + 1])
                    nc.vector.reciprocal_approx_fast(out=a_row[:], in_=a_row[:])
                    nc.vector.scalar_tensor_tensor(
                        b_row[:], mean[:], -1.0, a_row[:], ALU.mult, ALU.mult)
                    # broadcast rows across partitions
                    a_b = lp.tile([P, tok_len], F32, tag="ln_ab", bufs=1,
                                  name=f"ab_{out_tag}")
                    b_b = lp.tile([P, tok_len], F32, tag="ln_bb", bufs=1,
                                  name=f"bb_{out_tag}")
                    nc.gpsimd.partition_broadcast(a_b[:], a_row[:])
                    nc.gpsimd.partition_broadcast(b_b[:], b_row[:])
                    outs = []
                    for j in range(DT):
                        t1 = lp.tile([P, tok_len], F32, tag="ln_t1", bufs=2,
                                     name=f"t1_{j}_{out_tag}")
                        nc.vector.tensor_mul(t1[:], src[j][:], a_b[:])
                        nc.vector.tensor_add(t1[:], t1[:], b_b[:])
                        o = dst.tile([P, tok_len], out_dt, tag=f"{out_tag}{j}",
                                     name=f"{out_tag}{j}")
                        nc.scalar.activation(o[:], t1[:], AF.Identity,
                                             bias=bp[:, bcol + j:bcol + j + 1],
                                             scale=bp[:, gcol + j:gcol + j + 1])
                        outs.append(o)
                    return outs

            ssm_p, ssm_es = mkpool("ssm_p", "right")
            qkv_p, qkv_es = mkpool("qkv_p", "right")
            xn_p, xn_es = mkpool("xn_p", "right")
            gate_p, gate_es = mkpool("gate_p", "left")
            xn = layernorm(xt, T_TOT, L1G, L1B, BF16, "xn", xn_p)

            # ================= projections =================
            def load_w(dram_t, j, tag, bufs, cols=D):
                t = wpool.tile([P, cols], BF16, tag=tag, bufs=bufs,
                               name=f"{tag}_{dram_t.name}_{j}")
                nc.sync.dma_start(t[:], dram_t.ap()[j])
                return t

            def proj_T(wname, dram_t, tok0, tok1, bias_base, scale, out_dt,
                       douts=DT, out_len=None, pp=None, wcols=D, dst=None):
                """Transposed-out projection: out[dout_tile][P, tok1-tok0]."""
                outs = []
                wtiles = [load_w(dram_t, j, "wgt", 16, cols=wcols)
                          for j in range(DT)]
                out_len = out_len or (tok1 - tok0)
                for d in range(douts):
                    o = dst.tile([P, out_len], out_dt, tag=f"{wname}{d}",
                                 name=f"{wname}{d}")
                    for blk0 in range(0, out_len, 512):
                        blk1 = min(blk0 + 512, out_len)
                        ps = pp.tile([P, blk1 - blk0], F32, tag="proj", bufs=6,
                                     name=f"ps_{wname}{d}_{blk0}")
                        for j in range(DT):
                            nc.tensor.matmul(
                                ps[:], wtiles[j][:, d * P:(d + 1) * P],
                                xn[j][:, tok0 + blk0:tok0 + blk1],
                                start=(j == 0), stop=(j == DT - 1))
                        if bias_base is None:
                            nc.scalar.copy(o[:, blk0:blk1], ps[:])
                        else:
                            nc.scalar.activation(
                                o[:, blk0:blk1], ps[:], AF.Identity,
                                bias=bp[:, bias_base + d:bias_base + d + 1],
                                scale=scale)
                    outs.append(o)
                return outs

            with tc.tile_pool(name="ps_proj", bufs=1, space="PSUM") as pp:
                kt = proj_T("kt", wk, 0, T_TOT, BK, 1.0, BF16, pp=pp, dst=qkv_p)
                qt = proj_T("qt", wq, HALO, T_TOT, BQ, 0.125, BF16, pp=pp, dst=qkv_p)
                # u projection (SSM input), fp32 out, no bias
                ut = proj_T("ut", bw, 0, T_TOT, None, 1.0, F32, douts=2,
                            pp=pp, wcols=SSM_N, dst=ssm_p)
                # V in natural layout [tok_part, head*64]
                wvt = [load_w(wv, j, "wgt", 16) for j in range(DT)]
                vp = []
                for t_i in range(6):
                    v = qkv_p.tile([P, D], BF16, tag=f"vp{t_i}", name=f"vp{t_i}")
                    for b in range(2):
                        ps = pp.tile([P, 512], F32, tag="proj", bufs=6,
                                     name=f"ps_v{t_i}_{b}")
                        for j in range(DT):
                            nc.tensor.matmul(
                                ps[:], xn[j][:, t_i * P:(t_i + 1) * P],
                                wvt[j][:, b * 512:(b + 1) * 512],
                                start=(j == 0), stop=(j == DT - 1))
                        nc.scalar.copy(v[:, b * 512:(b + 1) * 512], ps[:])
                    vp.append(v)
                wgt_t = [load_w(wg, j, "wgt", 16) for j in range(DT)]
                gs = []
                for d in range(DT):
                    g = gate_p.tile([P, T_OWN], BF16, tag=f"gs{d}",
                                    name=f"gs{d}")
                    ps = pp.tile([P, T_OWN], F32, tag="proj", bufs=6,
                                 name=f"ps_g{d}")
                    for j in range(DT):
                        nc.tensor.matmul(ps[:], wgt_t[j][:, d * P:(d + 1) * P],
                                         xn[j][:, HALO:T_TOT],
                                         start=(j == 0), stop=(j == DT - 1))
                    nc.scalar.activation(g[:], ps[:], AF.Sigmoid,
                                         bias=bp[:, BG + d:BG + d + 1])
                    gs.append(g)
                xn_es.close()

            # ================= SSM scan =================
            states = []
            for i in range(2):
                # chunk-0 cores: zero the halo-u (nonzero via ln1_b), then
                # inject the initial state as a virtual token at halo col 255
                nc.vector.tensor_scalar_mul(
                    ut[i][:, 0:HALO], ut[i][:, 0:HALO],
                    bp[:, UHF:UHF + 1])
                nc.vector.tensor_scalar_add(
                    ut[i][:, HALO - 1:HALO], ut[i][:, HALO - 1:HALO],
                    bp[:, SINJ + i:SINJ + i + 1])
                ar = ssm_p.tile([P, T_TOT], F32, tag=f"ar{i}", name=f"ar{i}")
                nc.vector.memset(ar[:], 1.0)
                nc.vector.tensor_scalar_mul(ar[:], ar[:],
                                            bp[:, ALP + i:ALP + i + 1])
                st = ssm_p.tile([P, T_TOT], F32, tag=f"st{i}", name=f"st{i}")
                nc.vector.tensor_tensor_scan(st[:], ar[:], ut[i][:], 0.0,
                                             ALU.mult, ALU.add)
                nc.sync.dma_start(out_state.ap()[i], st[:, T_TOT - 1:T_TOT])
                sbf = ssm_p.tile([P, T_OWN], BF16, tag=f"sbf{i}", name=f"sbf{i}")
                nc.vector.tensor_copy(sbf[:], st[:, HALO:T_TOT])
                states.append(sbf)

            # ================= attention =================
            pt_p, pt_es = mkpool("pt_p", "right")
            aot_p, aot_es = mkpool("aot_p", "left")
            aot = [aot_p.tile([P, T_OWN], BF16, tag=f"aot{t}", name=f"aot{t}")
                   for t in range(DT)]
            pt = {}

            def scores_stage(j, pa):
                ptile = pt_p.tile([P, NH * NJ[j]], BF16, tag="pt", bufs=3,
                                padded_shape=[P, NH * 384], name=f"pt{j}")
                pt[j] = ptile
                n = NJ[j]
                for h in range(NH):
                    t, half = h // 2, h % 2
                    ps = pa.tile([P, n], F32, tag="sc", bufs=3,
                                 name=f"ps_sc{j}_{h}")
                    nc.tensor.matmul(
                        ps[:],
                        kt[t][half * DH:(half + 1) * DH, j * P:(j + 1) * P],
                        qt[t][half * DH:(half + 1) * DH, QS[j]:QE[j]],
                        start=True, stop=True)
                    nc.scalar.activation(ptile[:, h * n:(h + 1) * n], ps[:],
                                         AF.Exp)
                for hp in range(NH // 2):
                    sl = slice(2 * hp * n, (2 * hp + 2) * n)
                    nc.vector.tensor_mul(ptile[:, sl], ptile[:, sl],
                                         mk[:, MOFF[j]:MOFF[j] + 2 * n])

            def pv_stage(s, pa):
                for pr in range(8):
                    h0 = 2 * pr
                    pso = pa.tile([P, 2 * P], F32, tag="ao", bufs=3,
                                  name=f"ps_ao{s}_{pr}")
                    psd = pa.tile([P, 2 * P], F32, tag="den", bufs=2,
                                  name=f"ps_den{s}_{pr}")
                    for i, j in enumerate((s, s + 1, s + 2)):
                        n = NJ[j]
                        off = 128 * s - QS[j]
                        rhs = pt[j][:].rearrange("p (h q) -> p h q", q=n)[
                            :, h0:h0 + 2, off:off + P]
                        nc.tensor.matmul(pso[:], vp[j][:, DH * h0:DH * h0 + 128],
                                         rhs, start=(i == 0), stop=(i == 2))
                        nc.tensor.matmul(psd[:], ones_mat[:], rhs,
                                         start=(i == 0), stop=(i == 2))
                    recb = pt_p.tile([P, 2 * P], F32, tag="recb", bufs=3,
                                     name=f"recb{s}_{pr}")
                    nc.vector.reciprocal_approx_fast(out=recb[:], in_=psd[:])
                    nc.vector.tensor_mul(aot[pr][0:DH, 128 * s:128 * (s + 1)],
                                         pso[0:DH, 0:P], recb[0:DH, 0:P])
                    nc.vector.tensor_mul(aot[pr][DH:P, 128 * s:128 * (s + 1)],
                                         pso[DH:P, P:2 * P], recb[DH:P, P:2 * P])

            with tc.tile_pool(name="ps_attn", bufs=1, space="PSUM") as pa:
                # Vp slot for pair pr: cols [65*2pr, 65*2pr+130) -> slice 128 of it
                scores_stage(0, pa)
                scores_stage(1, pa)
                scores_stage(2, pa)
                for s in range(4):
                    pv_stage(s, pa)
                    if s + 3 <= 5:
                        scores_stage(s + 3, pa)
            pt_es.close()
            qkv_es.close()

            # ================= wo projection + y_ssm =================
            ya_p, ya_es = mkpool("ya_p", "left")
            with tc.tile_pool(name="ps_post", bufs=1, space="PSUM") as pp:
                cwtiles = [load_w(cw, i, "wgt", 16) for i in range(2)]
                ys = []
                for d in range(DT):
                    ps = pp.tile([P, T_OWN], F32, tag="proj", bufs=6,
                                 name=f"ps_cw{d}")
                    for i in range(2):
                        nc.tensor.matmul(ps[:], cwtiles[i][:, d * P:(d + 1) * P],
                                         states[i][:], start=(i == 0),
                                         stop=(i == 1))
                    y = ya_p.tile([P, T_OWN], F32, tag=f"ys{d}", name=f"ys{d}")
                    nc.scalar.copy(y[:], ps[:])
                    ys.append(y)

                wotiles = [load_w(wo, j, "wgt", 16) for j in range(DT)]
                ya = []
                for d in range(DT):
                    ps = pp.tile([P, T_OWN], F32, tag="proj", bufs=6,
                                 name=f"ps_wo{d}")
                    for t in range(DT):
                        nc.tensor.matmul(ps[:], wotiles[t][:, d * P:(d + 1) * P],
                                         aot[t][:], start=(t == 0),
                                         stop=(t == DT - 1))
                    y = ya_p.tile([P, T_OWN], F32, tag=f"ya{d}", name=f"ya{d}")
                    nc.scalar.activation(y[:], ps[:], AF.Identity,
                                         bias=bp[:, BO + d:BO + d + 1])
                    ya.append(y)
            # ================= gated fusion + residual =================
            xmid = []
            for d in range(DT):
                t1 = sb.tile([P, T_OWN], F32, tag="fus_t", bufs=3,
                             name=f"fus{d}")
                nc.vector.tensor_sub(t1[:], ya[d][:], ys[d][:])
                nc.vector.tensor_mul(t1[:], t1[:], gs[d][:])
                nc.vector.tensor_add(t1[:], t1[:], ys[d][:])
                xm = sb.tile([P, T_OWN], F32, tag=f"xmid{d}", name=f"xmid{d}")
                nc.vector.tensor_add(xm[:], t1[:], xt[d][:, HALO:T_TOT])
                xmid.append(xm)
            ssm_es.close()
            ya_es.close()
            aot_es.close()
            gate_es.close()
            x_es.close()

            # ================= LN2 + MLP =================
            with tc.tile_pool(name="mlp_p", bufs=1, side="left") as mlp_p:
                x3 = layernorm(xmid, T_OWN, L2G, L2B, BF16, "x3", mlp_p)
                with tc.tile_pool(name="ps_mlp", bufs=1, space="PSUM") as pm:
                    ht = []
                    for h in range(HT):
                        w1t = wpool.tile([P, D], BF16, tag="w1t", bufs=5,
                                         name=f"w1t{h}")
                        nc.sync.dma_start(
                            w1t[:].rearrange("p (j m) -> p j m", m=P),
                            w1.ap()[h].rearrange("j p m -> p j m"))
                        ps = pm.tile([P, T_OWN], F32, tag="h", bufs=3,
                                     name=f"ps_h{h}")
                        for j in range(DT):
                            nc.tensor.matmul(ps[:], w1t[:, j * P:(j + 1) * P],
                                             x3[j][:], start=(j == 0),
                                             stop=(j == DT - 1))
                        o = mlp_p.tile([P, T_OWN], BF16, tag="ht", bufs=HT,
                                       name=f"ht{h}")
                        nc.scalar.activation(o[:], ps[:], AF.Gelu,
                                             bias=bp[:, B1 + h:B1 + h + 1])
                        ht.append(o)
                    for half in range(2):
                        psd = [pm.tile([P, T_OWN], F32, tag=f"o{d4}", bufs=1,
                                       name=f"ps_o{half}_{d4}")
                               for d4 in range(4)]
                        for j in range(HT):
                            w2t = wpool.tile([P, 4 * P], BF16, tag="w2t",
                                             bufs=6, name=f"w2t{half}_{j}")
                            nc.sync.dma_start(w2t[:], w2.ap()[half, j])
                            for d4 in range(4):
                                nc.tensor.matmul(psd[d4][:],
                                                 w2t[:, d4 * P:(d4 + 1) * P],
                                                 ht[j][:], start=(j == 0),
                                                 stop=(j == HT - 1))
                        for d4 in range(4):
                            d = 4 * half + d4
                            o = mlp_p.tile([P, T_OWN], F32, tag="oT", bufs=3,
                                           name=f"oT{d}")
                            nc.vector.scalar_tensor_tensor(
                                o[:], psd[d4][:], bp[:, B2 + d:B2 + d + 1],
                                xmid[d][:], ALU.add, ALU.add)
                            nc.sync.dma_start(outT.ap()[d * P:(d + 1) * P, :],
                                              o[:])
    nc.compile()
    return nc


def _masks(chunk0: bool) -> np.ndarray:
    jj = np.arange(P)[:, None]
    mask = np.zeros((P, MOFF[6]), np.float32)
    for j in range(6):
        segs = []
        for s in range(max(0, j - 2), min(3, j) + 1):
            qq = np.arange(128)[None, :]
            if s == j:          # tri-A: key strictly after q
                m = (jj > qq)
            elif s == j - 1:    # fully valid
                m = np.ones((P, 128), bool)
            else:               # s == j - 2: causal
                m = (jj <= qq)
            segs.append(m.astype(np.float32))
        mj = np.concatenate(segs, axis=1)
        if chunk0 and j < 2:
            mj = np.zeros_like(mj)
        mask[:, MOFF[j]:MOFF[j] + 2 * NJ[j]] = np.concatenate([mj, mj], axis=1)
    return mask.astype(ml_dtypes.bfloat16)


def kernel(x, ssm_state, ln1_g, ln1_b, ln2_g, ln2_b, wq, bq, wk, bk, wv, bv,
           wo, bo, wg, bg, A, Bw, Cw, w1, b1, w2, b2):
    if "nc" not in _CACHE:
        _CACHE["nc"] = build()
    nc = _CACHE["nc"]

    bf = ml_dtypes.bfloat16

    def wl(w):      # [din, dout] -> [din/128, 128, dout] bf16
        return np.ascontiguousarray(w.reshape(DT, P, -1).astype(bf))

    wq_l, wk_l, wv_l, wg_l, wo_l = wl(wq), wl(wk), wl(wv), wl(wg), wl(wo)
    bw_l = np.ascontiguousarray(Bw.reshape(DT, P, SSM_N).astype(bf))
    cw_l = np.ascontiguousarray(Cw.reshape(2, P, D).astype(bf))
    w1_l = np.ascontiguousarray(
        w1.reshape(DT, P, HT, P).transpose(2, 0, 1, 3).astype(bf))
    w2_l = np.ascontiguousarray(
        w2.reshape(HT, P, 2, 512).transpose(2, 0, 1, 3).astype(bf))

    bo_eff = (bv.astype(np.float64) @ wo.astype(np.float64)
              + bo.astype(np.float64)).astype(np.float32)
    alpha = np.tanh(A.astype(np.float64)).astype(np.float32)

    bp = np.zeros((P, BPCOLS), np.float32)

    def fill(col, vec):
        v = vec.reshape(-1, P)
        for i in range(v.shape[0]):
            bp[:, col + i] = v[i]

    fill(BQ, bq * 0.125)
    fill(BK, bk)
    fill(BV, bv * 0)          # unused (folded into bo_eff)
    fill(BG, bg)
    fill(BO, bo_eff)
    fill(B1, b1)
    fill(B2, b2)
    fill(L1G, ln1_g)
    fill(L1B, ln1_b)
    fill(L2G, ln2_g)
    fill(L2B, ln2_b)
    fill(ALP, alpha)
    bp[:, EPSC] = EPS

    mask_c0 = _masks(True)
    mask_in = _masks(False)

    in_maps = []
    for c in range(8):
        b, s = c // 4, c % 4
        xfull = np.zeros((T_TOT, D), np.float32)
        t0 = s * T_OWN
        if s > 0:
            xfull[:HALO] = x[b, t0 - HALO:t0]
        xfull[HALO:] = x[b, t0:t0 + T_OWN]
        bpc = bp.copy()
        bpc[:, UHF] = 0.0 if s == 0 else 1.0
        if s == 0:
            st = np.asarray(ssm_state[b]).reshape(2, P)
            bpc[:, SINJ] = st[0]
            bpc[:, SINJ + 1] = st[1]
        in_maps.append({
            "xT": np.ascontiguousarray(xfull.T),
            "wq": wq_l, "wk": wk_l, "wv": wv_l, "wg": wg_l, "wo": wo_l,
            "bw": bw_l, "cw": cw_l, "w1": w1_l, "w2": w2_l,
            "biaspack": bpc,
            "maskpack": mask_c0 if s == 0 else mask_in,
        })

    res = run_bass_kernel_spmd(nc, in_maps, core_ids=list(range(8)))
    _CACHE["last_results"] = res

    x_out = np.zeros((2, 2048, D), np.float32)
    new_state = np.zeros((2, SSM_N), np.float32)
    for c in range(8):
        b, s = c // 4, c % 4
        x_out[b, s * T_OWN:(s + 1) * T_OWN] = res.results[c]["outT"].T
        if s == 3:
            new_state[b] = res.results[c]["out_state"].reshape(SSM_N)
    return x_out, new_state
